# revision 5
# baseline (speedup 1.0000x reference)
import numpy as np
import jax
import jax.numpy as jnp

# nn_ConditionalFlow: forward RQS flow log-det across 8 NeuronCores.
# Pure data parallel: batch N=262144 sharded 8 ways; tiny weights replicated.

N = 262144
DIMS_IN = 8
DIMS_C = 4
UNITS = 64
BINS = 10
N_PERMS = 3
N_BLOCKS = 2 * N_PERMS
MIN_BW = 0.001
MIN_BH = 0.001
MIN_D = 0.001
EPS = 1e-6
LOG2 = float(np.log(2.0))
NCORES = 8


def _build_masks():
    bits = np.array(
        [[int(ch) for ch in np.binary_repr(i, N_PERMS)] for i in range(DIMS_IN)]
    )
    bits = bits[:, ::-1].T
    m = np.repeat(bits, 2, axis=0).astype(bool)
    m[1::2] ^= True
    return m


MASKS = _build_masks()


def _rqs(inputs, uw, uh, ud):
    K = uw.shape[-1]
    ew = jnp.exp(uw - jnp.max(uw, axis=-1, keepdims=True))
    widths = MIN_BW + (1.0 - MIN_BW * K) * ew / jnp.sum(ew, axis=-1, keepdims=True)
    cw = jnp.cumsum(widths, axis=-1)
    cw = jnp.concatenate([jnp.zeros_like(cw[..., :1]), cw], axis=-1)
    cw = cw.at[..., -1].set(1.0)
    widths = cw[..., 1:] - cw[..., :-1]

    sp_in = 0.5 + 0.5 * jnp.exp(ud)
    deriv = (MIN_D + LOG2 + jnp.log(sp_in)) / (MIN_D + LOG2)

    eh = jnp.exp(uh - jnp.max(uh, axis=-1, keepdims=True))
    heights = MIN_BH + (1.0 - MIN_BH * K) * eh / jnp.sum(eh, axis=-1, keepdims=True)
    ch = jnp.cumsum(heights, axis=-1)
    ch = jnp.concatenate([jnp.zeros_like(ch[..., :1]), ch], axis=-1)
    ch = ch.at[..., -1].set(1.0)
    heights = ch[..., 1:] - ch[..., :-1]

    locs = cw.at[..., -1].add(EPS)
    bin_idx = jnp.clip(jnp.sum(inputs[..., None] >= locs, axis=-1) - 1, 0, K - 1)
    oh = (bin_idx[..., None] == jnp.arange(K)).astype(inputs.dtype)
    g = lambda a: jnp.sum(a[..., :K] * oh, axis=-1)

    in_cw = g(cw)
    in_bw = g(widths)
    in_ch = g(ch)
    delta = heights / widths
    in_delta = g(delta)
    in_d = g(deriv)
    in_dp1 = g(deriv[..., 1:])
    in_h = g(heights)

    theta = (inputs - in_cw) / in_bw
    tomt = theta * (1.0 - theta)
    numerator = in_h * (in_delta * theta**2 + in_d * tomt)
    denominator = in_delta + (in_d + in_dp1 - 2.0 * in_delta) * tomt
    outputs = in_ch + numerator / denominator
    deriv_num = in_delta**2 * (
        in_dp1 * theta**2 + 2.0 * in_delta * tomt + in_d * (1.0 - theta) ** 2
    )
    logabsdet = jnp.log(deriv_num) - 2.0 * jnp.log(denominator)
    return outputs, logabsdet


def _urqs(inputs, uw, uh, ud):
    inside = jnp.all((inputs >= 0.0) & (inputs <= 1.0), axis=-1)
    x_in = jnp.clip(inputs, 0.0, 1.0)
    o, lad = _rqs(x_in, uw, uh, ud)
    outputs = jnp.where(inside[:, None], o, inputs)
    logabsdet = jnp.where(inside[:, None], lad, 0.0)
    return outputs, logabsdet


def _flow_shard(x, c, W1, b1, W2, b2, W3, b3):
    jac = jnp.zeros((x.shape[0],), dtype=x.dtype)
    for i in range(N_BLOCKS):
        mask = MASKS[i]
        cond_idx = np.nonzero(mask)[0]
        trafo_idx = np.nonzero(~mask)[0]
        # static gather/scatter as permutation matmuls (axon XLA chokes on
        # dynamic-slice forms)
        Pc = np.zeros((DIMS_IN, 4), dtype=np.float32)
        Pc[cond_idx, np.arange(4)] = 1.0
        Pt = np.zeros((DIMS_IN, 4), dtype=np.float32)
        Pt[trafo_idx, np.arange(4)] = 1.0
        keep = jnp.asarray(mask.astype(np.float32))
        x_cond = jnp.concatenate([x @ jnp.asarray(Pc), c], axis=1)
        h = x_cond @ W1[i] + b1[i]
        h = 0.01 * h + 0.99 * jnp.maximum(h, 0.0)
        h = h @ W2[i] + b2[i]
        h = 0.01 * h + 0.99 * jnp.maximum(h, 0.0)
        out = (h @ W3[i] + b3[i]).reshape(x.shape[0], 4, 3 * BINS + 1)
        x_t = x @ jnp.asarray(Pt)
        x_out, bj = _urqs(
            x_t, out[..., :BINS], out[..., BINS : 2 * BINS], out[..., 2 * BINS :]
        )
        x = x * keep[None, :] + x_out @ jnp.asarray(Pt.T)
        jac = jac + bj.sum(axis=1)
    return jac


_pmapped = None


def _get_pmapped():
    global _pmapped
    if _pmapped is None:
        _pmapped = jax.pmap(
            _flow_shard,
            in_axes=(0, 0, None, None, None, None, None, None),
            devices=jax.devices()[:NCORES],
        )
    return _pmapped


def kernel(x, c, W1, b1, W2, b2, W3, b3):
    x = np.asarray(x, dtype=np.float32)
    c = np.asarray(c, dtype=np.float32)
    n = x.shape[0]
    per = n // NCORES
    xs = x.reshape(NCORES, per, DIMS_IN)
    cs = c.reshape(NCORES, per, DIMS_C)
    fn = _get_pmapped()
    jac = fn(
        xs,
        cs,
        jnp.asarray(W1),
        jnp.asarray(b1),
        jnp.asarray(W2),
        jnp.asarray(b2),
        jnp.asarray(W3),
        jnp.asarray(b3),
    )
    return np.asarray(jac).reshape(n).astype(np.float32)
